# revision 25
# baseline (speedup 1.0000x reference)
"""Per-pixel adaptive 5x5 conv (KPN) for Trainium2, 8-core data parallel.

out[g,h,w] = sum_{i,j} core[g,5i+j,h,w] * frames_pad[g,h+i-2,w+j-2]
with g = flattened (B,N) = 16 image planes; 2 planes per NeuronCore.

Engine split:
  DVE    : the 25 per-tap multiplies per image (fp16 taps in 2x_1P mode,
           fp8-weight taps at 1x), plus half the last image's PSUM casts.
  TensorE: all tap accumulation, via identity-matmul into PSUM (fp32
           accumulate; 4 matmuls of FD=512 per tap = 1 PSUM bank each).
  ScalarE: builds the odd-parity frame copy on-chip (saves 2.1 MB of HBM
           per core), img0's PSUM casts, output stores on the ACT ring.
  GpSimd : idle (it shares SBUF ports with DVE; offloading taps to it or
           routing bulk traffic through SWDGE measurably slows DVE).

Why it looks like this: the kernel is paced by DVE (25 x 1.22us fp16
muls per image) against the HBM weight stream through the 16 SDMA
engines, one of which runs ~17% slower than the rest and gates every
transfer-completion semaphore. So streamed bytes are minimized: tap
group 0 of each image ships as raw fp8-e4m3 (halves those bytes; the
whole-output rel err becomes ~1.2e-2, within the 2e-2 budget), the
odd-parity frame copy is built on-chip by the otherwise-idle scalar
engine, and outputs ship as fp16. Group 0 of img0 streams per-tap so
compute starts ~12.5us in; the last fp16 group streams per-tap so the
post-stream tail is one tap, not five; the final casts/stores pipeline
with the last taps' matmuls on alternating engines via per-bank PSUM
tiles. Deep weight/product pools decouple DVE from PE p-state dips.

Layout: rows interleaved 4-per-partition. Partition p holds padded rows
4p..4p+7 (= orig rows 4p-2..4p+5), so ALL row shifts i=0..4 are free-dim
offsets -- no cross-partition moves and no per-shift duplication. The
parity copy keeps every tap's 512-col slice 4-byte aligned for 2x mode.
  fin  [2, 128, 8*518] fp16: fin[img,p,row*518+col] = Fpad[img,4p+row,col+1]
  win8 [2, 128, 5*2048] fp8e4: tap group 0 in consumption order 0,2,4,1,3
  win  [2, 4, 128, 5*4*512] fp16: tap groups 1-4
  oout [2, 128, 4*512] fp16: oout[img,p,r*512+c] = out[img,4p+r,c]
"""

import os
import sys

import numpy as np

for _p in ("/opt/trn_rl_repo",):
    if _p not in sys.path and os.path.isdir(_p):
        sys.path.insert(0, _p)

K = 5
NCORES = 8
IMGS_PER_CORE = 2
H = W = 512
RPP = 4          # output rows per partition
FROWS = RPP + K - 1  # 8 padded rows held per partition
FCOLS = 518
FH_FREE = FROWS * FCOLS  # 4144 elems per parity copy
W_FREE = K * RPP * W  # 10240
T_FREE = RPP * W  # 2048 (one tap's weights / one product / output)
KORDER = [0, 2, 4, 1, 3]  # even-j taps first within each group
# fp8 taps, in consumption order: all of group 0
FP8_TAPS = [(0, 0), (0, 2), (0, 4), (0, 1), (0, 3)]
N8 = len(FP8_TAPS)

_compiled = {}
last_results = None  # BassKernelResults of the most recent run (for test.py)


def _build_nc():
    import concourse.bacc as bacc
    import concourse.mybir as mybir
    from concourse.tile import TileContext

    f16 = mybir.dt.float16
    f32 = mybir.dt.float32
    f8 = mybir.dt.float8e4

    nc = bacc.Bacc(None, target_bir_lowering=False, debug=False)
    fin = nc.dram_tensor("fin", [IMGS_PER_CORE, 128, FH_FREE], f16,
                         kind="ExternalInput")
    win8 = nc.dram_tensor("win8", [IMGS_PER_CORE, 128, N8 * T_FREE], f8,
                          kind="ExternalInput")
    win = nc.dram_tensor("win", [IMGS_PER_CORE, K - 1, 128, W_FREE], f16,
                         kind="ExternalInput")
    iden = nc.dram_tensor("iden", [128, 128], f16, kind="ExternalInput")
    oout = nc.dram_tensor("oout", [IMGS_PER_CORE, 128, T_FREE], f16,
                          kind="ExternalOutput")

    with TileContext(nc) as tc:
        with (
            tc.tile_pool(name="idp", bufs=1) as idp,
            tc.tile_pool(name="fpool", bufs=1) as fpool,
            tc.tile_pool(name="w8tap", bufs=5) as w8tap,
            tc.tile_pool(name="w8p", bufs=1) as w8p,
            tc.tile_pool(name="w8x", bufs=1) as w8x,
            tc.tile_pool(name="wgrp", bufs=4) as wgrp,
            tc.tile_pool(name="wtap", bufs=3) as wtap,
            tc.tile_pool(name="prpool", bufs=6) as prpool,
            tc.tile_pool(name="opool", bufs=1) as opool,
            tc.psum_pool(name="ppool", bufs=1) as ppool,
        ):
            id_t = idp.tile([128, 128], f16)
            nc.scalar.dma_start(out=id_t[:], in_=iden[:])

            for img in range(IMGS_PER_CORE):
                last_img = img == IMGS_PER_CORE - 1

                f0_t = fpool.tile([128, FH_FREE], f16, tag=f"f0_{img}")
                nc.sync.dma_start(out=f0_t[:], in_=fin[img])
                # on-chip odd-parity copy: par1[c] = par0[c-1]; col 0 unread
                f1_t = fpool.tile([128, FH_FREE], f16, tag=f"f1_{img}")
                nc.scalar.copy(out=f1_t[:, 1:FH_FREE],
                               in_=f0_t[:, 0:FH_FREE - 1])
                fviews = [
                    f0_t[:].rearrange("p (row col) -> p row col", col=FCOLS),
                    f1_t[:].rearrange("p (row col) -> p row col", col=FCOLS),
                ]

                if last_img:
                    # per-bank PSUM tiles so the final casts/stores can
                    # pipeline with the last tap's matmuls
                    psb = [ppool.tile([128, W], f32, tag=f"psb{b}",
                                      name=f"psb{b}")
                           for b in range(RPP)]
                    ps_out = lambda b: psb[b][:]
                else:
                    ps = ppool.tile([128, T_FREE], f32, tag=f"ps{img}")
                    ps_out = lambda b: ps[:, b * W:(b + 1) * W]

                # ---- build the 25-tap consumption schedule ----
                # taps: list of (wview, i, j); group 0 ships as raw fp8
                # (1x DVE mode, overlapped with the stream ramp)
                taps = []
                if img == 0:
                    # per-tap fp8 chunks: compute starts ~3.5us sooner.
                    # Taps 0-2 are consumed before the scalar engine could
                    # widen them (it is busy with the parity copy until
                    # ~16us), so they run fp8-direct at 1x; taps 3-4 are
                    # widened to fp16 right after the parity copy and run
                    # at 2x.
                    w8_ts = []
                    for c, (i, j) in enumerate(FP8_TAPS):
                        w_t = w8tap.tile([128, T_FREE], f8, tag="w8t",
                                         name="w8t")
                        nc.sync.dma_start(
                            out=w_t[:],
                            in_=win8[img][:, c * T_FREE:(c + 1) * T_FREE])
                        w8_ts.append(w_t)
                        if c < 3:
                            taps.append((w_t[:].rearrange(
                                "p (r c) -> p r c", r=RPP), i, j))
                    for c in (3, 4):
                        x_t = w8x.tile([128, T_FREE], f16, tag=f"w0x{c}",
                                       name="w0x")
                        nc.scalar.copy(out=x_t[:], in_=w8_ts[c][:])
                        taps.append((x_t[:].rearrange(
                            "p (r c) -> p r c", r=RPP), *FP8_TAPS[c]))
                    # hoist img1's fp8 chunk into the stream now and let the
                    # idle scalar engine widen it to fp16 mid-kernel, so
                    # img1's group-0 muls run at 2x instead of 1x
                    w8_t1 = w8p.tile([128, N8 * T_FREE], f8, tag="w8")
                    nc.sync.dma_start(out=w8_t1[:], in_=win8[1])
                    w8x_views = []
                    for c in range(N8):
                        x_t = w8x.tile([128, T_FREE], f16, tag=f"w8x{c}",
                                       name="w8x")
                        nc.scalar.copy(
                            out=x_t[:],
                            in_=w8_t1[:, c * T_FREE:(c + 1) * T_FREE])
                        w8x_views.append(x_t[:].rearrange(
                            "p (r c) -> p r c", r=RPP))
                else:
                    for c, (i, j) in enumerate(FP8_TAPS):
                        taps.append((w8x_views[c], i, j))

                for tg in range(1, K):
                    if last_img and tg == K - 1:
                        for k in KORDER:
                            w_t = wtap.tile([128, T_FREE], f16, tag="wt",
                                            name="wt")
                            nc.sync.dma_start(
                                out=w_t[:],
                                in_=win[img, tg - 1][:, k * T_FREE:(k + 1) * T_FREE])
                            taps.append((w_t[:].rearrange(
                                "p (r c) -> p r c", r=RPP), tg, k))
                    else:
                        w_t = wgrp.tile([128, W_FREE], f16, tag="wg")
                        nc.sync.dma_start(out=w_t[:], in_=win[img, tg - 1])
                        wv = w_t[:].rearrange("p (k r c) -> p k r c",
                                              k=K, r=RPP, c=W)
                        for k in KORDER:
                            taps.append((wv[:, k], tg, k))

                # ---- compute ----
                for n, (w_ap, i, j) in enumerate(taps):
                    par = j & 1
                    joff = j + par
                    prod = prpool.tile([128, T_FREE], f16, tag="pr")
                    pv = prod[:].rearrange("p (r c) -> p r c", r=RPP)
                    f_ap = fviews[par][:, i:i + RPP, joff:joff + W]
                    nc.vector.tensor_mul(out=pv, in0=w_ap, in1=f_ap)
                    for b in range(RPP):
                        nc.tensor.matmul(
                            ps_out(b),
                            id_t[:],
                            prod[:, b * W:(b + 1) * W],
                            start=(n == 0),
                            stop=(n == K * K - 1),
                        )

                o_t = opool.tile([128, T_FREE], f16, tag=f"o{img}")
                if last_img:
                    # pipeline: cast banks on alternating engines as each
                    # bank's accumulation completes; store in two halves
                    # on separate DGE rings
                    for b in range(RPP):
                        ob = o_t[:, b * W:(b + 1) * W]
                        if b % 2 == 0:
                            nc.vector.tensor_copy(out=ob, in_=ps_out(b))
                        else:
                            nc.scalar.copy(out=ob, in_=ps_out(b))
                        if b == 1:
                            nc.sync.dma_start(out=oout[img][:, :2 * W],
                                              in_=o_t[:, :2 * W])
                    nc.scalar.dma_start(out=oout[img][:, 2 * W:],
                                        in_=o_t[:, 2 * W:])
                else:
                    for b in range(RPP):
                        nc.scalar.copy(out=o_t[:, b * W:(b + 1) * W],
                                       in_=ps_out(b))
                    nc.scalar.dma_start(out=oout[img], in_=o_t[:])
    nc.finalize()
    return nc


def _host_prep(frames, core):
    """Build per-core in_maps. frames [4,4,1,512,512] f32, core [4,4,25,1,512,512]."""
    import concourse.mybir as mybir

    G = NCORES * IMGS_PER_CORE  # 16
    F = np.ascontiguousarray(frames.reshape(G, H, W))
    Wc = core.reshape(G, K * K, H, W)

    # frames: Fpad[g, R, C] = F[g, R-2, C-3]; rows pad 2/2, cols 3/4
    Fp = np.pad(F, ((0, 0), (2, 2), (3, 4))).astype(np.float16)  # [G,516,519]
    # 8-row windows starting at every 4th row: sw[g, p, row, col] = Fp[g, 4p+row, col]
    sw = np.lib.stride_tricks.sliding_window_view(Fp, FROWS, axis=1)
    sw = sw[:, ::RPP].transpose(0, 1, 3, 2)  # [G, 128, 8, 519]
    fprep = np.ascontiguousarray(sw[..., 1:1 + FCOLS])  # par=0: Fpad col c+1

    # weights: wall[g, t, p, r, c] = core[g, t, 4p+r, c]
    wall = Wc.reshape(G, K * K, 128, RPP, W)
    f8np = mybir.dt.np(mybir.dt.float8e4)

    w8 = np.stack([wall[:, 5 * i + j] for i, j in FP8_TAPS],
                  axis=2)  # [G, 128, 5, r, c]
    w8 = w8.astype(f8np)
    w16 = (wall[:, 5:].reshape(G, K - 1, K, 128, RPP, W)
           .transpose(0, 1, 3, 2, 4, 5).astype(np.float16))

    iden = np.eye(128, dtype=np.float16)
    in_maps = []
    for c in range(NCORES):
        g0 = c * IMGS_PER_CORE
        sl = slice(g0, g0 + IMGS_PER_CORE)
        in_maps.append({
            "fin": np.ascontiguousarray(
                fprep[sl].reshape(IMGS_PER_CORE, 128, FH_FREE)),
            "win8": np.ascontiguousarray(
                w8[sl].reshape(IMGS_PER_CORE, 128, N8 * T_FREE)),
            "win": np.ascontiguousarray(
                w16[sl].reshape(IMGS_PER_CORE, K - 1, 128, W_FREE)),
            "iden": iden,
        })
    return in_maps


def kernel(frames, core, bias):
    global last_results
    from concourse.bass_utils import run_bass_kernel_spmd

    frames = np.asarray(frames, dtype=np.float32)
    core = np.asarray(core, dtype=np.float32)

    if "nc" not in _compiled:
        _compiled["nc"] = _build_nc()
    nc = _compiled["nc"]

    in_maps = _host_prep(frames, core)
    trace = os.environ.get("KC_TRACE") == "1"
    tmpdir = os.environ.get("KC_TRACE_DIR") or None
    if tmpdir:
        os.makedirs(tmpdir, exist_ok=True)
    res = run_bass_kernel_spmd(nc, in_maps, list(range(NCORES)), trace=trace,
                               tmpdir=tmpdir)
    last_results = res

    G = NCORES * IMGS_PER_CORE
    out = np.empty((G, H, W), np.float32)
    for c in range(NCORES):
        o = res.results[c]["oout"]  # [2, 128, 2048] f16; rows are 4p+r in order
        for img in range(IMGS_PER_CORE):
            out[c * IMGS_PER_CORE + img] = o[img].reshape(H, W).astype(np.float32)
    return out.reshape(4, 4, H, W)


# revision 26
# speedup vs baseline: 1.0328x; 1.0328x over previous
"""Per-pixel adaptive 5x5 conv (KPN) for Trainium2, 8-core data parallel.

out[g,h,w] = sum_{i,j} core[g,5i+j,h,w] * frames_pad[g,h+i-2,w+j-2]
with g = flattened (B,N) = 16 image planes; 2 planes per NeuronCore.

Engine split:
  DVE    : the 25 per-tap multiplies per image (fp16 taps in 2x_1P mode,
           fp8-weight taps at 1x), plus half the last image's PSUM casts.
  TensorE: all tap accumulation, via identity-matmul into PSUM (fp32
           accumulate; 4 matmuls of FD=512 per tap = 1 PSUM bank each).
  ScalarE: builds the odd-parity frame copy on-chip (saves 2.1 MB of HBM
           per core), img0's PSUM casts, output stores on the ACT ring.
  GpSimd : idle (it shares SBUF ports with DVE; offloading taps to it or
           routing bulk traffic through SWDGE measurably slows DVE).

Why it looks like this: the kernel is paced by DVE (25 x 1.22us fp16
muls per image) against the HBM weight stream through the 16 SDMA
engines, one of which runs ~17% slower than the rest and gates every
transfer-completion semaphore. So streamed bytes are minimized: tap
group 0 of each image ships as raw fp8-e4m3 (halves those bytes; the
whole-output rel err becomes ~1.2e-2, within the 2e-2 budget), the
odd-parity frame copy is built on-chip by the otherwise-idle scalar
engine, and outputs ship as fp16. Group 0 of img0 streams per-tap so
compute starts ~12.5us in; the last fp16 group streams per-tap so the
post-stream tail is one tap, not five; the final casts/stores pipeline
with the last taps' matmuls on alternating engines via per-bank PSUM
tiles. Deep weight/product pools decouple DVE from PE p-state dips.

Layout: rows interleaved 4-per-partition. Partition p holds padded rows
4p..4p+7 (= orig rows 4p-2..4p+5), so ALL row shifts i=0..4 are free-dim
offsets -- no cross-partition moves and no per-shift duplication. The
parity copy keeps every tap's 512-col slice 4-byte aligned for 2x mode.
  fin  [2, 128, 8*518] fp16: fin[img,p,row*518+col] = Fpad[img,4p+row,col+1]
  win8 [2, 128, 5*2048] fp8e4: tap group 0 in consumption order 0,2,4,1,3
  win  [2, 4, 128, 5*4*512] fp16: tap groups 1-4
  oout [2, 128, 4*512] fp16: oout[img,p,r*512+c] = out[img,4p+r,c]
"""

import os
import sys

import numpy as np

for _p in ("/opt/trn_rl_repo",):
    if _p not in sys.path and os.path.isdir(_p):
        sys.path.insert(0, _p)

K = 5
NCORES = 8
IMGS_PER_CORE = 2
H = W = 512
RPP = 4          # output rows per partition
FROWS = RPP + K - 1  # 8 padded rows held per partition
FCOLS = 518
FH_FREE = FROWS * FCOLS  # 4144 elems per parity copy
W_FREE = K * RPP * W  # 10240
T_FREE = RPP * W  # 2048 (one tap's weights / one product / output)
KORDER = [0, 2, 4, 1, 3]  # even-j taps first within each group
# fp8 taps, in consumption order: all of group 0
FP8_TAPS = [(0, 0), (0, 2), (0, 4), (0, 1), (0, 3)]
N8 = len(FP8_TAPS)

_compiled = {}
last_results = None  # BassKernelResults of the most recent run (for test.py)


def _build_nc():
    import concourse.bacc as bacc
    import concourse.mybir as mybir
    from concourse.tile import TileContext

    f16 = mybir.dt.float16
    f32 = mybir.dt.float32
    f8 = mybir.dt.float8e4

    nc = bacc.Bacc(None, target_bir_lowering=False, debug=False)
    fin = nc.dram_tensor("fin", [IMGS_PER_CORE, 128, FH_FREE], f16,
                         kind="ExternalInput")
    win8 = nc.dram_tensor("win8", [IMGS_PER_CORE, 128, N8 * T_FREE], f8,
                          kind="ExternalInput")
    win = nc.dram_tensor("win", [IMGS_PER_CORE, K - 1, 128, W_FREE], f16,
                         kind="ExternalInput")
    iden = nc.dram_tensor("iden", [128, 128], f16, kind="ExternalInput")
    oout = nc.dram_tensor("oout", [IMGS_PER_CORE, 128, T_FREE], f16,
                          kind="ExternalOutput")

    with TileContext(nc) as tc:
        with (
            tc.tile_pool(name="idp", bufs=1) as idp,
            tc.tile_pool(name="fpool", bufs=1) as fpool,
            tc.tile_pool(name="w8tap", bufs=5) as w8tap,
            tc.tile_pool(name="w8p", bufs=1) as w8p,
            tc.tile_pool(name="w8x", bufs=1) as w8x,
            tc.tile_pool(name="wgrp", bufs=4) as wgrp,
            tc.tile_pool(name="wtap", bufs=5) as wtap,
            tc.tile_pool(name="prpool", bufs=6) as prpool,
            tc.tile_pool(name="opool", bufs=1) as opool,
            tc.psum_pool(name="ppool", bufs=1) as ppool,
        ):
            id_t = idp.tile([128, 128], f16)
            nc.scalar.dma_start(out=id_t[:], in_=iden[:])

            for img in range(IMGS_PER_CORE):
                last_img = img == IMGS_PER_CORE - 1

                f0_t = fpool.tile([128, FH_FREE], f16, tag=f"f0_{img}")
                nc.sync.dma_start(out=f0_t[:], in_=fin[img])
                # on-chip odd-parity copy: par1[c] = par0[c-1]; col 0 unread
                f1_t = fpool.tile([128, FH_FREE], f16, tag=f"f1_{img}")
                nc.scalar.copy(out=f1_t[:, 1:FH_FREE],
                               in_=f0_t[:, 0:FH_FREE - 1])
                fviews = [
                    f0_t[:].rearrange("p (row col) -> p row col", col=FCOLS),
                    f1_t[:].rearrange("p (row col) -> p row col", col=FCOLS),
                ]

                if last_img:
                    # per-bank PSUM tiles so the final casts/stores can
                    # pipeline with the last tap's matmuls
                    psb = [ppool.tile([128, W], f32, tag=f"psb{b}",
                                      name=f"psb{b}")
                           for b in range(RPP)]
                    ps_out = lambda b: psb[b][:]
                else:
                    ps = ppool.tile([128, T_FREE], f32, tag=f"ps{img}")
                    ps_out = lambda b: ps[:, b * W:(b + 1) * W]

                # ---- build the 25-tap consumption schedule ----
                # taps: list of (wview, i, j); group 0 ships as raw fp8
                # (1x DVE mode, overlapped with the stream ramp)
                taps = []
                if img == 0:
                    # per-tap fp8 chunks: compute starts ~3.5us sooner
                    for c, (i, j) in enumerate(FP8_TAPS):
                        w_t = w8tap.tile([128, T_FREE], f8, tag="w8t",
                                         name="w8t")
                        nc.sync.dma_start(
                            out=w_t[:],
                            in_=win8[img][:, c * T_FREE:(c + 1) * T_FREE])
                        taps.append((w_t[:].rearrange(
                            "p (r c) -> p r c", r=RPP), i, j))
                    # hoist img1's fp8 chunk into the stream now and let the
                    # idle scalar engine widen it to fp16 mid-kernel, so
                    # img1's group-0 muls run at 2x instead of 1x
                    w8_t1 = w8p.tile([128, N8 * T_FREE], f8, tag="w8")
                    nc.sync.dma_start(out=w8_t1[:], in_=win8[1])
                    w8x_views = []
                    for c in range(N8):
                        x_t = w8x.tile([128, T_FREE], f16, tag=f"w8x{c}",
                                       name="w8x")
                        nc.scalar.copy(
                            out=x_t[:],
                            in_=w8_t1[:, c * T_FREE:(c + 1) * T_FREE])
                        w8x_views.append(x_t[:].rearrange(
                            "p (r c) -> p r c", r=RPP))
                else:
                    for c, (i, j) in enumerate(FP8_TAPS):
                        taps.append((w8x_views[c], i, j))

                for tg in range(1, K):
                    if last_img and tg == K - 1:
                        for k in KORDER:
                            w_t = wtap.tile([128, T_FREE], f16, tag="wt",
                                            name="wt")
                            nc.sync.dma_start(
                                out=w_t[:],
                                in_=win[img, tg - 1][:, k * T_FREE:(k + 1) * T_FREE])
                            taps.append((w_t[:].rearrange(
                                "p (r c) -> p r c", r=RPP), tg, k))
                    else:
                        w_t = wgrp.tile([128, W_FREE], f16, tag="wg")
                        nc.sync.dma_start(out=w_t[:], in_=win[img, tg - 1])
                        wv = w_t[:].rearrange("p (k r c) -> p k r c",
                                              k=K, r=RPP, c=W)
                        for k in KORDER:
                            taps.append((wv[:, k], tg, k))

                # ---- compute ----
                for n, (w_ap, i, j) in enumerate(taps):
                    par = j & 1
                    joff = j + par
                    prod = prpool.tile([128, T_FREE], f16, tag="pr")
                    pv = prod[:].rearrange("p (r c) -> p r c", r=RPP)
                    f_ap = fviews[par][:, i:i + RPP, joff:joff + W]
                    nc.vector.tensor_mul(out=pv, in0=w_ap, in1=f_ap)
                    for b in range(RPP):
                        nc.tensor.matmul(
                            ps_out(b),
                            id_t[:],
                            prod[:, b * W:(b + 1) * W],
                            start=(n == 0),
                            stop=(n == K * K - 1),
                        )

                o_t = opool.tile([128, T_FREE], f16, tag=f"o{img}")
                if last_img:
                    # pipeline: cast banks on alternating engines as each
                    # bank's accumulation completes; store in two halves
                    # on separate DGE rings
                    for b in range(RPP):
                        ob = o_t[:, b * W:(b + 1) * W]
                        if b % 2 == 0:
                            nc.vector.tensor_copy(out=ob, in_=ps_out(b))
                        else:
                            nc.scalar.copy(out=ob, in_=ps_out(b))
                        if b == 1:
                            nc.sync.dma_start(out=oout[img][:, :2 * W],
                                              in_=o_t[:, :2 * W])
                    nc.scalar.dma_start(out=oout[img][:, 2 * W:],
                                        in_=o_t[:, 2 * W:])
                else:
                    for b in range(RPP):
                        nc.scalar.copy(out=o_t[:, b * W:(b + 1) * W],
                                       in_=ps_out(b))
                    nc.scalar.dma_start(out=oout[img], in_=o_t[:])
    nc.finalize()
    return nc


def _host_prep(frames, core):
    """Build per-core in_maps. frames [4,4,1,512,512] f32, core [4,4,25,1,512,512]."""
    import concourse.mybir as mybir

    G = NCORES * IMGS_PER_CORE  # 16
    F = np.ascontiguousarray(frames.reshape(G, H, W))
    Wc = core.reshape(G, K * K, H, W)

    # frames: Fpad[g, R, C] = F[g, R-2, C-3]; rows pad 2/2, cols 3/4
    Fp = np.pad(F, ((0, 0), (2, 2), (3, 4))).astype(np.float16)  # [G,516,519]
    # 8-row windows starting at every 4th row: sw[g, p, row, col] = Fp[g, 4p+row, col]
    sw = np.lib.stride_tricks.sliding_window_view(Fp, FROWS, axis=1)
    sw = sw[:, ::RPP].transpose(0, 1, 3, 2)  # [G, 128, 8, 519]
    fprep = np.ascontiguousarray(sw[..., 1:1 + FCOLS])  # par=0: Fpad col c+1

    # weights: wall[g, t, p, r, c] = core[g, t, 4p+r, c]
    wall = Wc.reshape(G, K * K, 128, RPP, W)
    f8np = mybir.dt.np(mybir.dt.float8e4)

    w8 = np.stack([wall[:, 5 * i + j] for i, j in FP8_TAPS],
                  axis=2)  # [G, 128, 5, r, c]
    w8 = w8.astype(f8np)
    w16 = (wall[:, 5:].reshape(G, K - 1, K, 128, RPP, W)
           .transpose(0, 1, 3, 2, 4, 5).astype(np.float16))

    iden = np.eye(128, dtype=np.float16)
    in_maps = []
    for c in range(NCORES):
        g0 = c * IMGS_PER_CORE
        sl = slice(g0, g0 + IMGS_PER_CORE)
        in_maps.append({
            "fin": np.ascontiguousarray(
                fprep[sl].reshape(IMGS_PER_CORE, 128, FH_FREE)),
            "win8": np.ascontiguousarray(
                w8[sl].reshape(IMGS_PER_CORE, 128, N8 * T_FREE)),
            "win": np.ascontiguousarray(
                w16[sl].reshape(IMGS_PER_CORE, K - 1, 128, W_FREE)),
            "iden": iden,
        })
    return in_maps


def kernel(frames, core, bias):
    global last_results
    from concourse.bass_utils import run_bass_kernel_spmd

    frames = np.asarray(frames, dtype=np.float32)
    core = np.asarray(core, dtype=np.float32)

    if "nc" not in _compiled:
        _compiled["nc"] = _build_nc()
    nc = _compiled["nc"]

    in_maps = _host_prep(frames, core)
    trace = os.environ.get("KC_TRACE") == "1"
    tmpdir = os.environ.get("KC_TRACE_DIR") or None
    if tmpdir:
        os.makedirs(tmpdir, exist_ok=True)
    res = run_bass_kernel_spmd(nc, in_maps, list(range(NCORES)), trace=trace,
                               tmpdir=tmpdir)
    last_results = res

    G = NCORES * IMGS_PER_CORE
    out = np.empty((G, H, W), np.float32)
    for c in range(NCORES):
        o = res.results[c]["oout"]  # [2, 128, 2048] f16; rows are 4p+r in order
        for img in range(IMGS_PER_CORE):
            out[c * IMGS_PER_CORE + img] = o[img].reshape(H, W).astype(np.float32)
    return out.reshape(4, 4, H, W)
